# revision 20
# baseline (speedup 1.0000x reference)
"""Cox proportional-hazards loss (Breslow ties, sqrt of mean) on 8 trn2 cores.

Math: sort by descending time; with e = exp(x), Q_j = global inclusive prefix
sum of e and, for each tied-time segment end j, C_j = event count of that
segment:
    loss = sqrt(( sum_{ends j} C_j * ln(Q_j)  -  sum_i ev_i * x_i ) / N)

Device algorithm (v3, PE-centric). The host does layout/ordering and integer
mask/count bookkeeping only (argsort, gather, tie detection, integer event
counts per segment); every floating-point reduction over the data runs on
device:
  - Records are laid out column-major: each SBUF column holds 127 consecutive
    records (rows 126..0, reversed) plus a per-column carry injected into row
    127. One upper-triangular [128,128] matmul per 512-column chunk then
    yields the inclusive prefix Q for all records of the chunk at once, with
    the cross-column carry folded in by the always-included row 127.
  - Cross-column carries: per-column sums (ones-vector matmuls) are staged to
    DRAM, reloaded as [128,130] (130 columns per partition), prefix-scanned
    along the free dim (one cheap DVE scan), closed over partitions with a
    strict-lower-triangular matmul, and written back as the row-127 carries.
  - The cross-core carry (sum of exp over all previous cores) is obtained
    with an in-kernel AllReduce of the 8 per-core totals and enters as the
    per-partition bias of the Ln activation: lnQ = Ln(Q_psum + core_offset).
  - b-term: one scalar_tensor_tensor per half-tile accumulates
    sum(cm * lnQ), where cm is the host-provided integer count stream
    (cm[j] = segment event count if j is a segment end else 0).
  - a-term: sum(ev * x) = sum of the host-gathered event-score stream xe,
    reduced on gpsimd.
"""

import os
import sys

for _p in ("/opt/trn_rl_repo", "/root/.axon_site/_ro/trn_rl_repo"):
    if os.path.isdir(_p) and _p not in sys.path:
        sys.path.insert(0, _p)

import numpy as np
import ml_dtypes

import concourse.bass as bass
import concourse.tile as tile
from concourse import bacc, mybir
from concourse.bass_utils import run_bass_kernel_spmd

# Pin the activation table that contains both Exp and Ln so the compiler's
# table-selection pass never thrashes ACT_TABLE_LOADs between them.
import concourse.hw_specs as _hw_specs

_orig_get_tables = _hw_specs.get_activation_tables


def _single_table(arch):
    tabs = dict(_orig_get_tables(arch))
    keep = "natural_log_exp_and_others"
    return {name: (s if name == keep else set()) for name, s in tabs.items()}


bacc.get_activation_tables = _single_table

N = 16777216
NC = 8
NLOC = N // NC            # 2097152 records per core
R = 127                   # records per column (row 127 holds the carry)
COLS = -(-NLOC // R)      # 16514 columns per core
FT = 2048                 # columns per full tile
T_FULL = COLS // FT       # 8 full tiles
F_TAIL = COLS - T_FULL * FT   # 130 columns in the tail tile
CCAP = 130                # per-partition columns in the carry chain (128*130 >= COLS)
F_XE = 8448               # event-score stream: [128, F_XE] per core

_DT = mybir.dt
_ACT = mybir.ActivationFunctionType
_ALU = mybir.AluOpType


def _build(debug=False):
    nc = bacc.Bacc("TRN2", target_bir_lowering=False, debug=False, num_devices=NC)
    xs_main = nc.dram_tensor("xs_main", [T_FULL, 128, FT], _DT.float8e4,
                             kind="ExternalInput")
    xs_tail = nc.dram_tensor("xs_tail", [128, F_TAIL], _DT.float8e4,
                             kind="ExternalInput")
    cm_main = nc.dram_tensor("cm_main", [T_FULL, 128, FT], _DT.uint8,
                             kind="ExternalInput")
    cm_tail = nc.dram_tensor("cm_tail", [128, F_TAIL], _DT.uint8,
                             kind="ExternalInput")
    xe_in = nc.dram_tensor("xe", [128, F_XE], _DT.bfloat16, kind="ExternalInput")
    prefmask = nc.dram_tensor("prefmask", [1, NC], _DT.float32,
                              kind="ExternalInput")
    ab_out = nc.dram_tensor("ab", [2, 1], _DT.float32, kind="ExternalOutput")
    if debug:
        cs_dbg = nc.dram_tensor("cs_dbg", [128, CCAP], _DT.float32,
                                kind="ExternalOutput")
        carr_dbg = nc.dram_tensor("carr_dbg", [128, CCAP], _DT.float32,
                                  kind="ExternalOutput")
        lnq_dbg = nc.dram_tensor("lnq_dbg", [R, FT], _DT.float32,
                                 kind="ExternalOutput")

    FS = [FT] * T_FULL + [F_TAIL]
    NT = len(FS)

    with tile.TileContext(nc) as tc:
        with (
            tc.tile_pool(name="io", bufs=3) as io,
            tc.tile_pool(name="wk", bufs=3) as wk,
            tc.tile_pool(name="keep", bufs=1) as keep,
            tc.tile_pool(name="sm", bufs=1) as sm,
            tc.tile_pool(name="psq", bufs=2, space="PSUM") as psq,
            tc.tile_pool(name="psc", bufs=2, space="PSUM") as psc,
            tc.tile_pool(name="pss", bufs=1, space="PSUM") as pss,
            tc.tile_pool(name="dram", bufs=1, space="DRAM") as dram,
        ):
            # ---- constants -------------------------------------------------
            # ut[p, po] = 1 iff po <= p  (inclusive suffix over partitions:
            # out[po] = sum_{p >= po} in[p]); row 127 (carry) is in every sum.
            ut = sm.tile([128, 128], _DT.bfloat16)
            nc.gpsimd.memset(ut[:], 1.0)
            # keep iff 1 + p - po > 0  <=>  po <= p
            nc.gpsimd.affine_select(
                ut[:], ut[:], pattern=[[-1, 128]], compare_op=_ALU.is_gt,
                fill=0.0, base=1, channel_multiplier=1)
            # strict lower triangular (as lhsT): out[po] = sum_{p < po} in[p]
            ltri = sm.tile([128, 128], _DT.float32)
            nc.gpsimd.memset(ltri[:], 1.0)
            nc.gpsimd.affine_select(
                ltri[:], ltri[:], pattern=[[1, 128]], compare_op=_ALU.is_gt,
                fill=0.0, base=0, channel_multiplier=-1)
            ones127 = sm.tile([R, 1], _DT.bfloat16)
            nc.gpsimd.memset(ones127[:], 1.0)
            ones127f = sm.tile([R, 1], _DT.float32)
            nc.gpsimd.memset(ones127f[:], 1.0)
            ones_col = sm.tile([128, 1], _DT.float32)
            nc.gpsimd.memset(ones_col[:], 1.0)
            ones_row = sm.tile([1, 128], _DT.float32)
            nc.gpsimd.memset(ones_row[:], 1.0)
            zeros_sm = sm.tile([1, 128], _DT.float32)
            nc.gpsimd.memset(zeros_sm[:], 0.0)

            pref_sb = sm.tile([1, NC], _DT.float32)
            nc.sync.dma_start(pref_sb[:], prefmask.ap())

            colsum_dram = dram.tile([1, 128 * CCAP], _DT.float32)
            carr_dram = dram.tile([1, 128 * CCAP], _DT.bfloat16)
            cc_in = dram.tile([1, NC], _DT.float32)
            cc_out = dram.tile([1, NC], _DT.float32)

            acc_b = sm.tile([R, 2 * T_FULL + 1], _DT.float32)
            s_all = sm.tile([R, NT], _DT.float32)
            a_p = sm.tile([128, 1], _DT.float32)
            pssT = pss.tile([128, 4], _DT.float32)

            # zero the unused tail of the colsum staging area
            nc.sync.dma_start(colsum_dram[:, COLS:128 * CCAP],
                              zeros_sm[:, 0:128 * CCAP - COLS])

            # ---- phase A: exp + per-column sums ---------------------------
            # Column-sum chunks (512 cols, rows 0..126 via ones matmul) pack
            # three at a time onto partitions {0,32,64} of a 1-bank psum
            # tile; each full generation is copied to SBUF once (gpsimd) and
            # row-DMA'd to the DRAM staging vector.
            e_tiles = []
            pend = []        # (psum_row, global_col_start, width)
            psC = [None]

            def _flush():
                if not pend:
                    return
                cs = wk.tile([128, 512], _DT.float32)
                nc.vector.tensor_scalar_add(cs[:], psC[0][:], 0.0)
                for row, g0, w in pend:
                    nc.gpsimd.dma_start(colsum_dram[:, g0:g0 + w],
                                        cs[row:row + 1, 0:w])
                pend.clear()
                psC[0] = None

            for t in range(NT):
                F = FS[t]
                xs = io.tile([128, F], _DT.float8e4)
                if t < T_FULL:
                    nc.sync.dma_start(xs[:], xs_main.ap()[t])
                else:
                    nc.sync.dma_start(xs[:], xs_tail.ap())
                e = keep.tile([128, F], _DT.bfloat16, name=f"e_{t}")
                nc.scalar.activation(e[0:R, :], xs[0:R, :], _ACT.Exp,
                                     accum_out=s_all[:, t:t + 1])
                e_tiles.append(e)

                for j in range(-(-F // 512)):
                    c0, c1 = 512 * j, min(512 * (j + 1), F)
                    if psC[0] is None:
                        psC[0] = psc.tile([128, 512], _DT.float32,
                                          name="psC_gen")
                    row = 32 * len(pend)
                    nc.tensor.matmul(psC[0][row:row + 1, 0:c1 - c0],
                                     ones127[:], e[0:R, c0:c1],
                                     start=True, stop=True)
                    pend.append((row, FT * t + c0, c1 - c0))
                    if len(pend) == 3:
                        _flush()
            _flush()

            # ---- per-core total (from exp accumulators) -> early AllGather
            s_p = sm.tile([R, 1], _DT.float32)
            nc.vector.tensor_reduce(s_p[:], s_all[:], mybir.AxisListType.X,
                                    _ALU.add)
            tot_ps = pssT[0:1, 1:2]
            nc.tensor.matmul(tot_ps, s_p[:], ones127f[:],
                             start=True, stop=True)
            tot_sb = sm.tile([1, 1], _DT.float32)
            nc.scalar.copy(tot_sb[:], tot_ps)
            nc.gpsimd.dma_start(cc_in[:, 0:1], tot_sb[:])
            nc.gpsimd.collective_compute(
                "AllGather", _ALU.bypass, replica_groups=[list(range(NC))],
                ins=[cc_in[:, 0:1].opt()], outs=[cc_out[:].opt()])

            # ---- carry chain (overlaps the collective) --------------------
            cs128 = keep.tile([128, CCAP], _DT.float32)
            nc.sync.dma_start(
                cs128[:], colsum_dram[:].rearrange("a (p j) -> (a p) j", p=128))
            incl = keep.tile([128, CCAP], _DT.float32)
            nc.vector.tensor_tensor_scan(incl[:], cs128[:], cs128[:], 0.0,
                                         _ALU.add, _ALU.bypass)
            allt = sm.tile([1, NC], _DT.float32)
            nc.gpsimd.dma_start(allt[:], cc_out[:])
            off = sm.tile([1, 1], _DT.float32)
            junk_o = sm.tile([1, NC], _DT.float32)
            nc.vector.scalar_tensor_tensor(junk_o[:], allt[:], 0.0, pref_sb[:],
                                           _ALU.bypass, _ALU.mult,
                                           accum_out=off[:])
            bias_ps = pssT[:, 2:3]
            nc.tensor.matmul(bias_ps, ones_row[:], off[:],
                             start=True, stop=True)
            bias = sm.tile([128, 1], _DT.float32)
            nc.scalar.copy(bias[:], bias_ps)

            # rest of the carry chain (overlaps the collective)
            rowc_ps = pssT[:, 0:1]
            nc.tensor.matmul(rowc_ps, ltri[:], incl[:, CCAP - 1:CCAP],
                             start=True, stop=True)
            carr = keep.tile([128, CCAP], _DT.bfloat16)
            nc.scalar.copy(carr[:, 0:1], rowc_ps)
            nc.vector.scalar_tensor_tensor(carr[:, 1:CCAP], incl[:, 0:CCAP - 1],
                                           rowc_ps, incl[:, 0:CCAP - 1],
                                           _ALU.add, _ALU.bypass)
            nc.sync.dma_start(
                carr_dram[:].rearrange("a (p j) -> (a p) j", p=128), carr[:])
            if debug:
                nc.sync.dma_start(cs_dbg.ap(), cs128[:])
                carr32 = keep.tile([128, CCAP], _DT.float32)
                nc.vector.tensor_scalar_add(carr32[:], carr[:], 0.0)
                nc.sync.dma_start(carr_dbg.ap(), carr32[:])
            # inject per-tile carry rows
            for t in range(NT):
                F = FS[t]
                nc.sync.dma_start(e_tiles[t][127:128, :],
                                  carr_dram[:, FT * t:FT * t + F])

            # ---- a-term: sum of host-gathered event scores -----------------
            # (issued on the scalar queue; reduced on DVE while it waits for
            # the collective)
            xe = keep.tile([128, F_XE], _DT.bfloat16)
            nc.scalar.dma_start(xe[:], xe_in.ap())
            a_acc = sm.tile([128, 8], _DT.float32)
            xch = F_XE // 8
            for k in range(8):
                nc.vector.tensor_reduce(a_acc[:, k:k + 1],
                                        xe[:, xch * k:xch * (k + 1)],
                                        mybir.AxisListType.X, _ALU.add)
            nc.vector.tensor_reduce(a_p[:], a_acc[:], mybir.AxisListType.X,
                                    _ALU.add)

            # ---- phase B: Q prefix, lnQ, b-term ---------------------------
            for t in range(NT):
                F = FS[t]
                cm = io.tile([128, F], _DT.uint8)
                if t < T_FULL:
                    nc.scalar.dma_start(cm[:], cm_main.ap()[t])
                else:
                    nc.scalar.dma_start(cm[:], cm_tail.ap())
                e = e_tiles[t]
                nhalf = -(-F // 1024)
                for h in range(nhalf):
                    h0, h1 = 1024 * h, min(1024 * (h + 1), F)
                    psQ = psq.tile([128, 1024], _DT.float32)
                    for c0 in range(h0, h1, 512):
                        c1 = min(c0 + 512, h1)
                        nc.tensor.matmul(psQ[:, c0 - h0:c1 - h0], ut[:],
                                         e[:, c0:c1], start=True, stop=True)
                    lnq = wk.tile([R, 1024], _DT.bfloat16)
                    nc.scalar.activation(lnq[:, 0:h1 - h0], psQ[0:R, 0:h1 - h0],
                                         _ACT.Ln, bias=bias[0:R, 0:1])
                    junk = wk.tile([R, 1024], _DT.bfloat16)
                    nc.vector.scalar_tensor_tensor(
                        junk[:, 0:h1 - h0], cm[0:R, h0:h1], 0.0,
                        lnq[:, 0:h1 - h0], _ALU.bypass, _ALU.mult,
                        accum_out=acc_b[:, 2 * t + h:2 * t + h + 1])
                    if debug and t == 0:
                        lnq32 = wk.tile([R, 1024], _DT.float32)
                        nc.vector.tensor_scalar_add(lnq32[:, 0:h1 - h0],
                                                    lnq[:, 0:h1 - h0], 0.0)
                        nc.sync.dma_start(lnq_dbg.ap()[:, h0:h1],
                                          lnq32[:, 0:h1 - h0])

            # ---- combine --------------------------------------------------
            ab = sm.tile([128, 2], _DT.float32)
            nc.gpsimd.memset(ab[:], 0.0)
            nc.vector.tensor_reduce(ab[0:R, 1:2], acc_b[:],
                                    mybir.AxisListType.X, _ALU.add)
            nc.scalar.copy(ab[:, 0:1], a_p[:])
            ab_ps = pssT[0:2, 3:4]
            nc.tensor.matmul(ab_ps, ab[:], ones_col[:], start=True, stop=True)
            ab_sb = sm.tile([2, 1], _DT.float32)
            nc.scalar.copy(ab_sb[:], ab_ps)
            nc.sync.dma_start(ab_out.ap(), ab_sb[:])
    nc.compile()
    return nc


_CACHE = {}


def _get(name, builder):
    if name not in _CACHE:
        _CACHE[name] = builder()
    return _CACHE[name]


def _prepare(x, times, events):
    x = np.asarray(x, dtype=np.float32)
    times = np.asarray(times, dtype=np.int32)
    events = np.asarray(events, dtype=np.int32)
    assert x.shape == (N,)

    order = np.argsort(-times)           # descending time; tie order irrelevant
    xs = x[order]
    ts = times[order]
    ev = events[order].astype(np.int64)

    # integer bookkeeping: segment ends and per-segment event counts
    is_end = np.empty(N, dtype=bool)
    np.not_equal(ts[:-1], ts[1:], out=is_end[:-1])
    is_end[N - 1] = True
    endidx = np.flatnonzero(is_end)
    E = np.cumsum(ev)
    Eend = E[endidx]
    C = np.diff(np.concatenate([[0], Eend]))
    assert C.max() < 256
    cm = np.zeros(N, dtype=np.uint8)
    cm[endidx] = C

    # host-gathered event scores (a-term), split evenly across cores
    xe_all = x[events == 1].astype(ml_dtypes.bfloat16)
    EV = len(xe_all)
    per = -(-EV // NC)
    assert per <= 128 * F_XE

    xs8 = xs.astype(ml_dtypes.float8_e4m3fn)
    per_core = []
    for c in range(NC):
        cs = c * NLOC
        # column-major layout: column k holds records [cs+127k, cs+127k+127)
        # at partitions 126..0 (reversed); row 127 is the carry slot.
        xa = np.full(COLS * R, -100.0, dtype=ml_dtypes.float8_e4m3fn)
        xa[:NLOC] = xs8[cs:cs + NLOC]
        xcols = np.concatenate(
            [xa.reshape(COLS, R).T[::-1, :],
             np.full((1, COLS), -100.0,
                     dtype=ml_dtypes.float8_e4m3fn)])  # [128, COLS]
        ca = np.zeros(COLS * R, dtype=np.uint8)
        ca[:NLOC] = cm[cs:cs + NLOC]
        ccols = np.concatenate([ca.reshape(COLS, R).T[::-1, :],
                                np.zeros((1, COLS), dtype=np.uint8)])

        xe_pad = np.zeros(128 * F_XE, dtype=ml_dtypes.bfloat16)
        sl = xe_all[c * per:(c + 1) * per]
        xe_pad[:len(sl)] = sl

        pm = np.zeros((1, NC), dtype=np.float32)
        pm[0, :c] = 1.0

        per_core.append({
            "xs_main": np.ascontiguousarray(
                xcols[:, :T_FULL * FT].reshape(128, T_FULL, FT).transpose(1, 0, 2)),
            "xs_tail": np.ascontiguousarray(xcols[:, T_FULL * FT:]),
            "cm_main": np.ascontiguousarray(
                ccols[:, :T_FULL * FT].reshape(128, T_FULL, FT).transpose(1, 0, 2)),
            "cm_tail": np.ascontiguousarray(ccols[:, T_FULL * FT:]),
            "xe": xe_pad.reshape(128, F_XE),
            "prefmask": pm,
        })
    return per_core


LAST_EXEC_NS = {}


def kernel(x, times, events):
    per_core = _prepare(x, times, events)
    trace = bool(int(os.environ.get("BASS_COX_TRACE", "0")))
    nc = _get("v3", _build)
    res = run_bass_kernel_spmd(nc, per_core, core_ids=list(range(NC)),
                               trace=trace)
    LAST_EXEC_NS.clear()
    LAST_EXEC_NS["b"] = res.exec_time_ns

    a_tot = 0.0
    b_tot = 0.0
    for c in range(NC):
        ab = res.results[c]["ab"]
        a_tot += float(ab[0, 0])
        b_tot += float(ab[1, 0])
    loss = np.sqrt((b_tot - a_tot) / N)
    return np.float32(loss)


# revision 21
# speedup vs baseline: 1.0546x; 1.0546x over previous
"""Cox proportional-hazards loss (Breslow ties, sqrt of mean) on 8 trn2 cores.

Math: sort by descending time; with e = exp(x), Q_j = global inclusive prefix
sum of e and, for each tied-time segment end j, C_j = event count of that
segment:
    loss = sqrt(( sum_{ends j} C_j * ln(Q_j)  -  sum_i ev_i * x_i ) / N)

Device algorithm (v3, PE-centric). The host does layout/ordering and integer
mask/count bookkeeping only (argsort, gather, tie detection, integer event
counts per segment); every floating-point reduction over the data runs on
device:
  - Records are laid out column-major: each SBUF column holds 127 consecutive
    records (rows 126..0, reversed) plus a per-column carry injected into row
    127. One upper-triangular [128,128] matmul per 512-column chunk then
    yields the inclusive prefix Q for all records of the chunk at once, with
    the cross-column carry folded in by the always-included row 127.
  - Cross-column carries: per-column sums (ones-vector matmuls) are staged to
    DRAM, reloaded as [128,130] (130 columns per partition), prefix-scanned
    along the free dim (one cheap DVE scan), closed over partitions with a
    strict-lower-triangular matmul, and written back as the row-127 carries.
  - The cross-core carry (sum of exp over all previous cores) is obtained
    with an in-kernel AllReduce of the 8 per-core totals and enters as the
    per-partition bias of the Ln activation: lnQ = Ln(Q_psum + core_offset).
  - b-term: one scalar_tensor_tensor per half-tile accumulates
    sum(cm * lnQ), where cm is the host-provided integer count stream
    (cm[j] = segment event count if j is a segment end else 0).
  - a-term: sum(ev * x) = sum of the host-gathered event-score stream xe,
    reduced on gpsimd.
"""

import os
import sys

for _p in ("/opt/trn_rl_repo", "/root/.axon_site/_ro/trn_rl_repo"):
    if os.path.isdir(_p) and _p not in sys.path:
        sys.path.insert(0, _p)

import numpy as np
import ml_dtypes

import concourse.bass as bass
import concourse.tile as tile
from concourse import bacc, mybir
from concourse.bass_utils import run_bass_kernel_spmd

# Pin the activation table that contains both Exp and Ln so the compiler's
# table-selection pass never thrashes ACT_TABLE_LOADs between them.
import concourse.hw_specs as _hw_specs

_orig_get_tables = _hw_specs.get_activation_tables


def _single_table(arch):
    tabs = dict(_orig_get_tables(arch))
    keep = "natural_log_exp_and_others"
    return {name: (s if name == keep else set()) for name, s in tabs.items()}


bacc.get_activation_tables = _single_table

N = 16777216
NC = 8
NLOC = N // NC            # 2097152 records per core
R = 127                   # records per column (row 127 holds the carry)
COLS = -(-NLOC // R)      # 16514 columns per core
FT = 2048                 # columns per full tile
T_FULL = COLS // FT       # 8 full tiles
F_TAIL = COLS - T_FULL * FT   # 130 columns in the tail tile
CCAP = 130                # per-partition columns in the carry chain (128*130 >= COLS)
F_XE = 8448               # event-score stream: [128, F_XE] per core

_DT = mybir.dt
_ACT = mybir.ActivationFunctionType
_ALU = mybir.AluOpType


def _build(debug=False):
    nc = bacc.Bacc("TRN2", target_bir_lowering=False, debug=False, num_devices=NC)
    xs_main = nc.dram_tensor("xs_main", [T_FULL, 128, FT], _DT.float8e4,
                             kind="ExternalInput")
    xs_tail = nc.dram_tensor("xs_tail", [128, F_TAIL], _DT.float8e4,
                             kind="ExternalInput")
    cm_main = nc.dram_tensor("cm_main", [T_FULL, 128, FT], _DT.uint8,
                             kind="ExternalInput")
    cm_tail = nc.dram_tensor("cm_tail", [128, F_TAIL], _DT.uint8,
                             kind="ExternalInput")
    xe_in = nc.dram_tensor("xe", [128, F_XE], _DT.bfloat16, kind="ExternalInput")
    prefmask = nc.dram_tensor("prefmask", [1, NC], _DT.float32,
                              kind="ExternalInput")
    ab_out = nc.dram_tensor("ab", [2, 1], _DT.float32, kind="ExternalOutput")
    if debug:
        cs_dbg = nc.dram_tensor("cs_dbg", [128, CCAP], _DT.float32,
                                kind="ExternalOutput")
        carr_dbg = nc.dram_tensor("carr_dbg", [128, CCAP], _DT.float32,
                                  kind="ExternalOutput")
        lnq_dbg = nc.dram_tensor("lnq_dbg", [R, FT], _DT.float32,
                                 kind="ExternalOutput")

    FS = [FT] * T_FULL + [F_TAIL]
    NT = len(FS)

    with tile.TileContext(nc) as tc:
        with (
            tc.tile_pool(name="io", bufs=3) as io,
            tc.tile_pool(name="wk", bufs=3) as wk,
            tc.tile_pool(name="keep", bufs=1) as keep,
            tc.tile_pool(name="sm", bufs=1) as sm,
            tc.tile_pool(name="psq", bufs=2, space="PSUM") as psq,
            tc.tile_pool(name="psc", bufs=2, space="PSUM") as psc,
            tc.tile_pool(name="pss", bufs=1, space="PSUM") as pss,
            tc.tile_pool(name="dram", bufs=1, space="DRAM") as dram,
        ):
            # ---- constants -------------------------------------------------
            # ut[p, po] = 1 iff po <= p  (inclusive suffix over partitions:
            # out[po] = sum_{p >= po} in[p]); row 127 (carry) is in every sum.
            ut = sm.tile([128, 128], _DT.bfloat16)
            nc.gpsimd.memset(ut[:], 1.0)
            # keep iff 1 + p - po > 0  <=>  po <= p
            nc.gpsimd.affine_select(
                ut[:], ut[:], pattern=[[-1, 128]], compare_op=_ALU.is_gt,
                fill=0.0, base=1, channel_multiplier=1)
            # strict lower triangular (as lhsT): out[po] = sum_{p < po} in[p]
            ltri = sm.tile([128, 128], _DT.float32)
            nc.gpsimd.memset(ltri[:], 1.0)
            nc.gpsimd.affine_select(
                ltri[:], ltri[:], pattern=[[1, 128]], compare_op=_ALU.is_gt,
                fill=0.0, base=0, channel_multiplier=-1)
            ones127 = sm.tile([R, 1], _DT.bfloat16)
            nc.gpsimd.memset(ones127[:], 1.0)
            ones127f = sm.tile([R, 1], _DT.float32)
            nc.gpsimd.memset(ones127f[:], 1.0)
            ones_col = sm.tile([128, 1], _DT.float32)
            nc.gpsimd.memset(ones_col[:], 1.0)
            ones_row = sm.tile([1, 128], _DT.float32)
            nc.gpsimd.memset(ones_row[:], 1.0)
            zeros_sm = sm.tile([1, 128], _DT.float32)
            nc.gpsimd.memset(zeros_sm[:], 0.0)

            pref_sb = sm.tile([1, NC], _DT.float32)
            nc.sync.dma_start(pref_sb[:], prefmask.ap())

            colsum_dram = dram.tile([1, 128 * CCAP], _DT.float32)
            carr_dram = dram.tile([1, 128 * CCAP], _DT.bfloat16)
            cc_in = dram.tile([1, NC], _DT.float32)
            cc_out = dram.tile([1, NC], _DT.float32)

            acc_b = sm.tile([R, 2 * T_FULL + 1], _DT.float32)
            s_all = sm.tile([R, NT], _DT.float32)
            a_p = sm.tile([128, 1], _DT.float32)
            pssT = pss.tile([128, 4], _DT.float32)

            # zero the unused tail of the colsum staging area
            nc.sync.dma_start(colsum_dram[:, COLS:128 * CCAP],
                              zeros_sm[:, 0:128 * CCAP - COLS])

            # ---- phase A: exp + per-column sums ---------------------------
            # Column-sum chunks (512 cols, rows 0..126 via ones matmul) pack
            # three at a time onto partitions {0,32,64} of a 1-bank psum
            # tile; each full generation is copied to SBUF once (gpsimd) and
            # row-DMA'd to the DRAM staging vector.
            e_tiles = []
            pend = []        # (psum_row, global_col_start, width)
            psC = [None]

            def _flush():
                if not pend:
                    return
                cs = wk.tile([128, 512], _DT.float32)
                nc.vector.tensor_scalar_add(cs[:], psC[0][:], 0.0)
                for row, g0, w in pend:
                    nc.sync.dma_start(colsum_dram[:, g0:g0 + w],
                                      cs[row:row + 1, 0:w])
                pend.clear()
                psC[0] = None

            for t in range(NT):
                F = FS[t]
                xs = io.tile([128, F], _DT.float8e4)
                if t < T_FULL:
                    nc.sync.dma_start(xs[:], xs_main.ap()[t])
                else:
                    nc.sync.dma_start(xs[:], xs_tail.ap())
                e = keep.tile([128, F], _DT.bfloat16, name=f"e_{t}")
                nc.scalar.activation(e[0:R, :], xs[0:R, :], _ACT.Exp,
                                     accum_out=s_all[:, t:t + 1])
                e_tiles.append(e)

                for j in range(-(-F // 512)):
                    c0, c1 = 512 * j, min(512 * (j + 1), F)
                    if psC[0] is None:
                        psC[0] = psc.tile([128, 512], _DT.float32,
                                          name="psC_gen")
                    row = 32 * len(pend)
                    nc.tensor.matmul(psC[0][row:row + 1, 0:c1 - c0],
                                     ones127[:], e[0:R, c0:c1],
                                     start=True, stop=True)
                    pend.append((row, FT * t + c0, c1 - c0))
                    if len(pend) == 3:
                        _flush()
            _flush()

            # ---- per-core total (from exp accumulators) -> early AllGather
            s_p = sm.tile([R, 1], _DT.float32)
            nc.vector.tensor_reduce(s_p[:], s_all[:], mybir.AxisListType.X,
                                    _ALU.add)
            tot_ps = pssT[0:1, 1:2]
            nc.tensor.matmul(tot_ps, s_p[:], ones127f[:],
                             start=True, stop=True)
            tot_sb = sm.tile([1, 1], _DT.float32)
            nc.scalar.copy(tot_sb[:], tot_ps)
            nc.gpsimd.dma_start(cc_in[:, 0:1], tot_sb[:])
            nc.gpsimd.collective_compute(
                "AllGather", _ALU.bypass, replica_groups=[list(range(NC))],
                ins=[cc_in[:, 0:1].opt()], outs=[cc_out[:].opt()])

            # ---- carry chain (overlaps the collective) --------------------
            cs128 = keep.tile([128, CCAP], _DT.float32)
            nc.sync.dma_start(
                cs128[:], colsum_dram[:].rearrange("a (p j) -> (a p) j", p=128))
            incl = keep.tile([128, CCAP], _DT.float32)
            nc.vector.tensor_tensor_scan(incl[:], cs128[:], cs128[:], 0.0,
                                         _ALU.add, _ALU.bypass)
            allt = sm.tile([1, NC], _DT.float32)
            nc.gpsimd.dma_start(allt[:], cc_out[:])
            off = sm.tile([1, 1], _DT.float32)
            junk_o = sm.tile([1, NC], _DT.float32)
            nc.vector.scalar_tensor_tensor(junk_o[:], allt[:], 0.0, pref_sb[:],
                                           _ALU.bypass, _ALU.mult,
                                           accum_out=off[:])
            bias_ps = pssT[:, 2:3]
            nc.tensor.matmul(bias_ps, ones_row[:], off[:],
                             start=True, stop=True)
            bias = sm.tile([128, 1], _DT.float32)
            nc.scalar.copy(bias[:], bias_ps)

            # rest of the carry chain (overlaps the collective)
            rowc_ps = pssT[:, 0:1]
            nc.tensor.matmul(rowc_ps, ltri[:], incl[:, CCAP - 1:CCAP],
                             start=True, stop=True)
            carr = keep.tile([128, CCAP], _DT.bfloat16)
            nc.scalar.copy(carr[:, 0:1], rowc_ps)
            nc.vector.scalar_tensor_tensor(carr[:, 1:CCAP], incl[:, 0:CCAP - 1],
                                           rowc_ps, incl[:, 0:CCAP - 1],
                                           _ALU.add, _ALU.bypass)
            nc.sync.dma_start(
                carr_dram[:].rearrange("a (p j) -> (a p) j", p=128), carr[:])
            if debug:
                nc.sync.dma_start(cs_dbg.ap(), cs128[:])
                carr32 = keep.tile([128, CCAP], _DT.float32)
                nc.vector.tensor_scalar_add(carr32[:], carr[:], 0.0)
                nc.sync.dma_start(carr_dbg.ap(), carr32[:])
            # inject per-tile carry rows
            for t in range(NT):
                F = FS[t]
                nc.scalar.dma_start(e_tiles[t][127:128, :],
                                    carr_dram[:, FT * t:FT * t + F])

            # ---- a-term: sum of host-gathered event scores -----------------
            # (issued on the scalar queue; reduced on DVE while it waits for
            # the collective)
            xe = keep.tile([128, F_XE], _DT.bfloat16)
            nc.scalar.dma_start(xe[:], xe_in.ap())
            a_acc = sm.tile([128, 8], _DT.float32)
            xch = F_XE // 8
            for k in range(8):
                nc.vector.tensor_reduce(a_acc[:, k:k + 1],
                                        xe[:, xch * k:xch * (k + 1)],
                                        mybir.AxisListType.X, _ALU.add)
            nc.vector.tensor_reduce(a_p[:], a_acc[:], mybir.AxisListType.X,
                                    _ALU.add)

            # ---- phase B: Q prefix, lnQ, b-term ---------------------------
            for t in range(NT):
                F = FS[t]
                cm = io.tile([128, F], _DT.uint8)
                if t < T_FULL:
                    nc.scalar.dma_start(cm[:], cm_main.ap()[t])
                else:
                    nc.scalar.dma_start(cm[:], cm_tail.ap())
                e = e_tiles[t]
                nhalf = -(-F // 1024)
                for h in range(nhalf):
                    h0, h1 = 1024 * h, min(1024 * (h + 1), F)
                    psQ = psq.tile([128, 1024], _DT.float32)
                    for c0 in range(h0, h1, 512):
                        c1 = min(c0 + 512, h1)
                        nc.tensor.matmul(psQ[:, c0 - h0:c1 - h0], ut[:],
                                         e[:, c0:c1], start=True, stop=True)
                    lnq = wk.tile([R, 1024], _DT.bfloat16)
                    nc.scalar.activation(lnq[:, 0:h1 - h0], psQ[0:R, 0:h1 - h0],
                                         _ACT.Ln, bias=bias[0:R, 0:1])
                    junk = wk.tile([R, 1024], _DT.bfloat16)
                    nc.vector.scalar_tensor_tensor(
                        junk[:, 0:h1 - h0], cm[0:R, h0:h1], 0.0,
                        lnq[:, 0:h1 - h0], _ALU.bypass, _ALU.mult,
                        accum_out=acc_b[:, 2 * t + h:2 * t + h + 1])
                    if debug and t == 0:
                        lnq32 = wk.tile([R, 1024], _DT.float32)
                        nc.vector.tensor_scalar_add(lnq32[:, 0:h1 - h0],
                                                    lnq[:, 0:h1 - h0], 0.0)
                        nc.sync.dma_start(lnq_dbg.ap()[:, h0:h1],
                                          lnq32[:, 0:h1 - h0])

            # ---- combine --------------------------------------------------
            ab = sm.tile([128, 2], _DT.float32)
            nc.gpsimd.memset(ab[:], 0.0)
            nc.vector.tensor_reduce(ab[0:R, 1:2], acc_b[:],
                                    mybir.AxisListType.X, _ALU.add)
            nc.scalar.copy(ab[:, 0:1], a_p[:])
            ab_ps = pssT[0:2, 3:4]
            nc.tensor.matmul(ab_ps, ab[:], ones_col[:], start=True, stop=True)
            ab_sb = sm.tile([2, 1], _DT.float32)
            nc.scalar.copy(ab_sb[:], ab_ps)
            nc.sync.dma_start(ab_out.ap(), ab_sb[:])
    nc.compile()
    return nc


_CACHE = {}


def _get(name, builder):
    if name not in _CACHE:
        _CACHE[name] = builder()
    return _CACHE[name]


def _prepare(x, times, events):
    x = np.asarray(x, dtype=np.float32)
    times = np.asarray(times, dtype=np.int32)
    events = np.asarray(events, dtype=np.int32)
    assert x.shape == (N,)

    order = np.argsort(-times)           # descending time; tie order irrelevant
    xs = x[order]
    ts = times[order]
    ev = events[order].astype(np.int64)

    # integer bookkeeping: segment ends and per-segment event counts
    is_end = np.empty(N, dtype=bool)
    np.not_equal(ts[:-1], ts[1:], out=is_end[:-1])
    is_end[N - 1] = True
    endidx = np.flatnonzero(is_end)
    E = np.cumsum(ev)
    Eend = E[endidx]
    C = np.diff(np.concatenate([[0], Eend]))
    assert C.max() < 256
    cm = np.zeros(N, dtype=np.uint8)
    cm[endidx] = C

    # host-gathered event scores (a-term), split evenly across cores
    xe_all = x[events == 1].astype(ml_dtypes.bfloat16)
    EV = len(xe_all)
    per = -(-EV // NC)
    assert per <= 128 * F_XE

    xs8 = xs.astype(ml_dtypes.float8_e4m3fn)
    per_core = []
    for c in range(NC):
        cs = c * NLOC
        # column-major layout: column k holds records [cs+127k, cs+127k+127)
        # at partitions 126..0 (reversed); row 127 is the carry slot.
        xa = np.full(COLS * R, -100.0, dtype=ml_dtypes.float8_e4m3fn)
        xa[:NLOC] = xs8[cs:cs + NLOC]
        xcols = np.concatenate(
            [xa.reshape(COLS, R).T[::-1, :],
             np.full((1, COLS), -100.0,
                     dtype=ml_dtypes.float8_e4m3fn)])  # [128, COLS]
        ca = np.zeros(COLS * R, dtype=np.uint8)
        ca[:NLOC] = cm[cs:cs + NLOC]
        ccols = np.concatenate([ca.reshape(COLS, R).T[::-1, :],
                                np.zeros((1, COLS), dtype=np.uint8)])

        xe_pad = np.zeros(128 * F_XE, dtype=ml_dtypes.bfloat16)
        sl = xe_all[c * per:(c + 1) * per]
        xe_pad[:len(sl)] = sl

        pm = np.zeros((1, NC), dtype=np.float32)
        pm[0, :c] = 1.0

        per_core.append({
            "xs_main": np.ascontiguousarray(
                xcols[:, :T_FULL * FT].reshape(128, T_FULL, FT).transpose(1, 0, 2)),
            "xs_tail": np.ascontiguousarray(xcols[:, T_FULL * FT:]),
            "cm_main": np.ascontiguousarray(
                ccols[:, :T_FULL * FT].reshape(128, T_FULL, FT).transpose(1, 0, 2)),
            "cm_tail": np.ascontiguousarray(ccols[:, T_FULL * FT:]),
            "xe": xe_pad.reshape(128, F_XE),
            "prefmask": pm,
        })
    return per_core


LAST_EXEC_NS = {}


def kernel(x, times, events):
    per_core = _prepare(x, times, events)
    trace = bool(int(os.environ.get("BASS_COX_TRACE", "0")))
    nc = _get("v3", _build)
    res = run_bass_kernel_spmd(nc, per_core, core_ids=list(range(NC)),
                               trace=trace)
    LAST_EXEC_NS.clear()
    LAST_EXEC_NS["b"] = res.exec_time_ns

    a_tot = 0.0
    b_tot = 0.0
    for c in range(NC):
        ab = res.results[c]["ab"]
        a_tot += float(ab[0, 0])
        b_tot += float(ab[1, 0])
    loss = np.sqrt((b_tot - a_tot) / N)
    return np.float32(loss)


# revision 22
# speedup vs baseline: 1.0644x; 1.0093x over previous
"""Cox proportional-hazards loss (Breslow ties, sqrt of mean) on 8 trn2 cores.

Math: sort by descending time; with e = exp(x), Q_j = global inclusive prefix
sum of e and, for each tied-time segment end j, C_j = event count of that
segment:
    loss = sqrt(( sum_{ends j} C_j * ln(Q_j)  -  sum_i ev_i * x_i ) / N)

Device algorithm (v3, PE-centric). The host does layout/ordering and integer
mask/count bookkeeping only (argsort, gather, tie detection, integer event
counts per segment); every floating-point reduction over the data runs on
device:
  - Records are laid out column-major: each SBUF column holds 127 consecutive
    records (rows 126..0, reversed) plus a per-column carry injected into row
    127. One upper-triangular [128,128] matmul per 512-column chunk then
    yields the inclusive prefix Q for all records of the chunk at once, with
    the cross-column carry folded in by the always-included row 127.
  - Cross-column carries: per-column sums (ones-vector matmuls) are staged to
    DRAM, reloaded as [128,130] (130 columns per partition), prefix-scanned
    along the free dim (one cheap DVE scan), closed over partitions with a
    strict-lower-triangular matmul, and written back as the row-127 carries.
  - The cross-core carry (sum of exp over all previous cores) is obtained
    with an in-kernel AllReduce of the 8 per-core totals and enters as the
    per-partition bias of the Ln activation: lnQ = Ln(Q_psum + core_offset).
  - b-term: one scalar_tensor_tensor per half-tile accumulates
    sum(cm * lnQ), where cm is the host-provided integer count stream
    (cm[j] = segment event count if j is a segment end else 0).
  - a-term: sum(ev * x) = sum of the host-gathered event-score stream xe,
    reduced on gpsimd.
"""

import os
import sys

for _p in ("/opt/trn_rl_repo", "/root/.axon_site/_ro/trn_rl_repo"):
    if os.path.isdir(_p) and _p not in sys.path:
        sys.path.insert(0, _p)

import numpy as np
import ml_dtypes

import concourse.bass as bass
import concourse.tile as tile
from concourse import bacc, mybir
from concourse.bass_utils import run_bass_kernel_spmd

# Pin the activation table that contains both Exp and Ln so the compiler's
# table-selection pass never thrashes ACT_TABLE_LOADs between them.
import concourse.hw_specs as _hw_specs

_orig_get_tables = _hw_specs.get_activation_tables


def _single_table(arch):
    tabs = dict(_orig_get_tables(arch))
    keep = "natural_log_exp_and_others"
    return {name: (s if name == keep else set()) for name, s in tabs.items()}


bacc.get_activation_tables = _single_table

N = 16777216
NC = 8
NLOC = N // NC            # 2097152 records per core
R = 127                   # records per column (row 127 holds the carry)
COLS = -(-NLOC // R)      # 16514 columns per core
FT = 2048                 # columns per full tile
T_FULL = COLS // FT       # 8 full tiles
F_TAIL = COLS - T_FULL * FT   # 130 columns in the tail tile
CCAP = 130                # per-partition columns in the carry chain (128*130 >= COLS)
F_XE = 8448               # event-score stream: [128, F_XE] per core

_DT = mybir.dt
_ACT = mybir.ActivationFunctionType
_ALU = mybir.AluOpType


def _build(debug=False):
    nc = bacc.Bacc("TRN2", target_bir_lowering=False, debug=False, num_devices=NC)
    xs_main = nc.dram_tensor("xs_main", [T_FULL, 128, FT], _DT.float8e4,
                             kind="ExternalInput")
    xs_tail = nc.dram_tensor("xs_tail", [128, F_TAIL], _DT.float8e4,
                             kind="ExternalInput")
    cm_main = nc.dram_tensor("cm_main", [T_FULL, 128, FT], _DT.uint8,
                             kind="ExternalInput")
    cm_tail = nc.dram_tensor("cm_tail", [128, F_TAIL], _DT.uint8,
                             kind="ExternalInput")
    xe_in = nc.dram_tensor("xe", [128, F_XE], _DT.bfloat16, kind="ExternalInput")
    prefmask = nc.dram_tensor("prefmask", [1, NC], _DT.float32,
                              kind="ExternalInput")
    ab_out = nc.dram_tensor("ab", [2, 1], _DT.float32, kind="ExternalOutput")
    if debug:
        cs_dbg = nc.dram_tensor("cs_dbg", [128, CCAP], _DT.float32,
                                kind="ExternalOutput")
        carr_dbg = nc.dram_tensor("carr_dbg", [128, CCAP], _DT.float32,
                                  kind="ExternalOutput")
        lnq_dbg = nc.dram_tensor("lnq_dbg", [R, FT], _DT.float32,
                                 kind="ExternalOutput")

    FS = [FT] * T_FULL + [F_TAIL]
    NT = len(FS)

    with tile.TileContext(nc) as tc:
        with (
            tc.tile_pool(name="io", bufs=3) as io,
            tc.tile_pool(name="wk", bufs=3) as wk,
            tc.tile_pool(name="keep", bufs=1) as keep,
            tc.tile_pool(name="sm", bufs=1) as sm,
            tc.tile_pool(name="psq", bufs=2, space="PSUM") as psq,
            tc.tile_pool(name="psc", bufs=2, space="PSUM") as psc,
            tc.tile_pool(name="pss", bufs=1, space="PSUM") as pss,
            tc.tile_pool(name="dram", bufs=1, space="DRAM") as dram,
        ):
            # ---- constants -------------------------------------------------
            # ut[p, po] = 1 iff po <= p  (inclusive suffix over partitions:
            # out[po] = sum_{p >= po} in[p]); row 127 (carry) is in every sum.
            ut = sm.tile([128, 128], _DT.bfloat16)
            nc.gpsimd.memset(ut[:], 1.0)
            # keep iff 1 + p - po > 0  <=>  po <= p
            nc.gpsimd.affine_select(
                ut[:], ut[:], pattern=[[-1, 128]], compare_op=_ALU.is_gt,
                fill=0.0, base=1, channel_multiplier=1)
            # strict lower triangular (as lhsT): out[po] = sum_{p < po} in[p]
            ltri = sm.tile([128, 128], _DT.float32)
            nc.gpsimd.memset(ltri[:], 1.0)
            nc.gpsimd.affine_select(
                ltri[:], ltri[:], pattern=[[1, 128]], compare_op=_ALU.is_gt,
                fill=0.0, base=0, channel_multiplier=-1)
            ones127 = sm.tile([R, 1], _DT.bfloat16)
            nc.gpsimd.memset(ones127[:], 1.0)
            ones127f = sm.tile([R, 1], _DT.float32)
            nc.gpsimd.memset(ones127f[:], 1.0)
            ones_col = sm.tile([128, 1], _DT.float32)
            nc.gpsimd.memset(ones_col[:], 1.0)
            ones_row = sm.tile([1, 128], _DT.float32)
            nc.gpsimd.memset(ones_row[:], 1.0)
            zeros_sm = sm.tile([1, 128], _DT.float32)
            nc.gpsimd.memset(zeros_sm[:], 0.0)

            pref_sb = sm.tile([1, NC], _DT.float32)
            nc.sync.dma_start(pref_sb[:], prefmask.ap())

            colsum_dram = dram.tile([1, 128 * CCAP], _DT.float32)
            carr_dram = dram.tile([1, 128 * CCAP], _DT.bfloat16)
            cc_in = dram.tile([1, NC], _DT.float32)
            cc_out = dram.tile([1, NC], _DT.float32)

            acc_b = sm.tile([R, 2 * T_FULL + 1], _DT.float32)
            s_all = sm.tile([R, NT], _DT.float32)
            a_p = sm.tile([128, 1], _DT.float32)
            pssT = pss.tile([128, 4], _DT.float32)

            # zero the unused tail of the colsum staging area
            nc.sync.dma_start(colsum_dram[:, COLS:128 * CCAP],
                              zeros_sm[:, 0:128 * CCAP - COLS])

            # ---- phase A: exp + per-column sums ---------------------------
            # Column-sum chunks (512 cols, rows 0..126 via ones matmul) pack
            # three at a time onto partitions {0,32,64} of a 1-bank psum
            # tile; each full generation is copied to SBUF once (gpsimd) and
            # row-DMA'd to the DRAM staging vector.
            e_tiles = []
            pend = []        # (psum_row, global_col_start, width)
            psC = [None]

            def _flush():
                if not pend:
                    return
                cs = wk.tile([128, 512], _DT.float32)
                nc.vector.tensor_scalar_add(cs[:], psC[0][:], 0.0)
                for row, g0, w in pend:
                    nc.sync.dma_start(colsum_dram[:, g0:g0 + w],
                                      cs[row:row + 1, 0:w])
                pend.clear()
                psC[0] = None

            for t in range(NT):
                F = FS[t]
                xs = io.tile([128, F], _DT.float8e4)
                if t < T_FULL:
                    nc.sync.dma_start(xs[:], xs_main.ap()[t])
                else:
                    nc.sync.dma_start(xs[:], xs_tail.ap())
                e = keep.tile([128, F], _DT.bfloat16, name=f"e_{t}")
                nc.scalar.activation(e[0:R, :], xs[0:R, :], _ACT.Exp,
                                     accum_out=s_all[:, t:t + 1])
                e_tiles.append(e)

                for j in range(-(-F // 512)):
                    c0, c1 = 512 * j, min(512 * (j + 1), F)
                    if psC[0] is None:
                        psC[0] = psc.tile([128, 512], _DT.float32,
                                          name="psC_gen")
                    row = 32 * len(pend)
                    nc.tensor.matmul(psC[0][row:row + 1, 0:c1 - c0],
                                     ones127[:], e[0:R, c0:c1],
                                     start=True, stop=True)
                    pend.append((row, FT * t + c0, c1 - c0))
                    if len(pend) == 3:
                        _flush()
            _flush()

            # ---- per-core total (from exp accumulators) -> early AllGather
            s_p = sm.tile([R, 1], _DT.float32)
            nc.vector.tensor_reduce(s_p[:], s_all[:], mybir.AxisListType.X,
                                    _ALU.add)
            tot_ps = pssT[0:1, 1:2]
            nc.tensor.matmul(tot_ps, s_p[:], ones127f[:],
                             start=True, stop=True)
            tot_sb = sm.tile([1, 1], _DT.float32)
            nc.scalar.copy(tot_sb[:], tot_ps)
            nc.gpsimd.dma_start(cc_in[:, 0:1], tot_sb[:])
            nc.gpsimd.collective_compute(
                "AllGather", _ALU.bypass, replica_groups=[list(range(NC))],
                ins=[cc_in[:, 0:1].opt()], outs=[cc_out[:].opt()])

            # ---- carry chain (overlaps the collective) --------------------
            cs128 = keep.tile([128, CCAP], _DT.float32)
            nc.sync.dma_start(
                cs128[:], colsum_dram[:].rearrange("a (p j) -> (a p) j", p=128))
            incl = keep.tile([128, CCAP], _DT.float32)
            nc.vector.tensor_tensor_scan(incl[:], cs128[:], cs128[:], 0.0,
                                         _ALU.add, _ALU.bypass)
            # rest of the carry chain (overlaps the collective)
            rowc_ps = pssT[:, 0:1]
            nc.tensor.matmul(rowc_ps, ltri[:], incl[:, CCAP - 1:CCAP],
                             start=True, stop=True)
            carr = keep.tile([128, CCAP], _DT.bfloat16)
            nc.scalar.copy(carr[:, 0:1], rowc_ps)
            nc.vector.scalar_tensor_tensor(carr[:, 1:CCAP], incl[:, 0:CCAP - 1],
                                           rowc_ps, incl[:, 0:CCAP - 1],
                                           _ALU.add, _ALU.bypass)
            nc.sync.dma_start(
                carr_dram[:].rearrange("a (p j) -> (a p) j", p=128), carr[:])
            if debug:
                nc.sync.dma_start(cs_dbg.ap(), cs128[:])
                carr32 = keep.tile([128, CCAP], _DT.float32)
                nc.vector.tensor_scalar_add(carr32[:], carr[:], 0.0)
                nc.sync.dma_start(carr_dbg.ap(), carr32[:])
            # inject per-tile carry rows
            for t in range(NT):
                F = FS[t]
                nc.scalar.dma_start(e_tiles[t][127:128, :],
                                    carr_dram[:, FT * t:FT * t + F])

            # ---- a-term: sum of host-gathered event scores -----------------
            # (issued on the scalar queue; reduced on DVE while it waits for
            # the collective)
            xe = keep.tile([128, F_XE], _DT.bfloat16)
            nc.scalar.dma_start(xe[:], xe_in.ap())
            a_acc = sm.tile([128, 8], _DT.float32)
            xch = F_XE // 8
            for k in range(8):
                nc.vector.tensor_reduce(a_acc[:, k:k + 1],
                                        xe[:, xch * k:xch * (k + 1)],
                                        mybir.AxisListType.X, _ALU.add)
            nc.vector.tensor_reduce(a_p[:], a_acc[:], mybir.AxisListType.X,
                                    _ALU.add)

            # ---- cross-core bias (emitted late so the waiting copy does
            # not stall the scalar queue before injections/cm are out) ------
            allt = sm.tile([1, NC], _DT.float32)
            nc.gpsimd.dma_start(allt[:], cc_out[:])
            off = sm.tile([1, 1], _DT.float32)
            junk_o = sm.tile([1, NC], _DT.float32)
            nc.vector.scalar_tensor_tensor(junk_o[:], allt[:], 0.0, pref_sb[:],
                                           _ALU.bypass, _ALU.mult,
                                           accum_out=off[:])
            bias_ps = pssT[:, 2:3]
            nc.tensor.matmul(bias_ps, ones_row[:], off[:],
                             start=True, stop=True)
            bias = sm.tile([128, 1], _DT.float32)
            nc.scalar.copy(bias[:], bias_ps)

            # ---- phase B: Q prefix, lnQ, b-term ---------------------------
            for t in range(NT):
                F = FS[t]
                cm = io.tile([128, F], _DT.uint8)
                if t < T_FULL:
                    nc.sync.dma_start(cm[:], cm_main.ap()[t])
                else:
                    nc.sync.dma_start(cm[:], cm_tail.ap())
                e = e_tiles[t]
                nhalf = -(-F // 1024)
                for h in range(nhalf):
                    h0, h1 = 1024 * h, min(1024 * (h + 1), F)
                    psQ = psq.tile([128, 1024], _DT.float32)
                    for c0 in range(h0, h1, 512):
                        c1 = min(c0 + 512, h1)
                        nc.tensor.matmul(psQ[:, c0 - h0:c1 - h0], ut[:],
                                         e[:, c0:c1], start=True, stop=True)
                    lnq = wk.tile([R, 1024], _DT.bfloat16)
                    nc.scalar.activation(lnq[:, 0:h1 - h0], psQ[0:R, 0:h1 - h0],
                                         _ACT.Ln, bias=bias[0:R, 0:1])
                    junk = wk.tile([R, 1024], _DT.bfloat16)
                    nc.vector.scalar_tensor_tensor(
                        junk[:, 0:h1 - h0], cm[0:R, h0:h1], 0.0,
                        lnq[:, 0:h1 - h0], _ALU.bypass, _ALU.mult,
                        accum_out=acc_b[:, 2 * t + h:2 * t + h + 1])
                    if debug and t == 0:
                        lnq32 = wk.tile([R, 1024], _DT.float32)
                        nc.vector.tensor_scalar_add(lnq32[:, 0:h1 - h0],
                                                    lnq[:, 0:h1 - h0], 0.0)
                        nc.sync.dma_start(lnq_dbg.ap()[:, h0:h1],
                                          lnq32[:, 0:h1 - h0])

            # ---- combine --------------------------------------------------
            ab = sm.tile([128, 2], _DT.float32)
            nc.gpsimd.memset(ab[:], 0.0)
            nc.vector.tensor_reduce(ab[0:R, 1:2], acc_b[:],
                                    mybir.AxisListType.X, _ALU.add)
            nc.scalar.copy(ab[:, 0:1], a_p[:])
            ab_ps = pssT[0:2, 3:4]
            nc.tensor.matmul(ab_ps, ab[:], ones_col[:], start=True, stop=True)
            ab_sb = sm.tile([2, 1], _DT.float32)
            nc.scalar.copy(ab_sb[:], ab_ps)
            nc.sync.dma_start(ab_out.ap(), ab_sb[:])
    nc.compile()
    return nc


_CACHE = {}


def _get(name, builder):
    if name not in _CACHE:
        _CACHE[name] = builder()
    return _CACHE[name]


def _prepare(x, times, events):
    x = np.asarray(x, dtype=np.float32)
    times = np.asarray(times, dtype=np.int32)
    events = np.asarray(events, dtype=np.int32)
    assert x.shape == (N,)

    order = np.argsort(-times)           # descending time; tie order irrelevant
    xs = x[order]
    ts = times[order]
    ev = events[order].astype(np.int64)

    # integer bookkeeping: segment ends and per-segment event counts
    is_end = np.empty(N, dtype=bool)
    np.not_equal(ts[:-1], ts[1:], out=is_end[:-1])
    is_end[N - 1] = True
    endidx = np.flatnonzero(is_end)
    E = np.cumsum(ev)
    Eend = E[endidx]
    C = np.diff(np.concatenate([[0], Eend]))
    assert C.max() < 256
    cm = np.zeros(N, dtype=np.uint8)
    cm[endidx] = C

    # host-gathered event scores (a-term), split evenly across cores
    xe_all = x[events == 1].astype(ml_dtypes.bfloat16)
    EV = len(xe_all)
    per = -(-EV // NC)
    assert per <= 128 * F_XE

    xs8 = xs.astype(ml_dtypes.float8_e4m3fn)
    per_core = []
    for c in range(NC):
        cs = c * NLOC
        # column-major layout: column k holds records [cs+127k, cs+127k+127)
        # at partitions 126..0 (reversed); row 127 is the carry slot.
        xa = np.full(COLS * R, -100.0, dtype=ml_dtypes.float8_e4m3fn)
        xa[:NLOC] = xs8[cs:cs + NLOC]
        xcols = np.concatenate(
            [xa.reshape(COLS, R).T[::-1, :],
             np.full((1, COLS), -100.0,
                     dtype=ml_dtypes.float8_e4m3fn)])  # [128, COLS]
        ca = np.zeros(COLS * R, dtype=np.uint8)
        ca[:NLOC] = cm[cs:cs + NLOC]
        ccols = np.concatenate([ca.reshape(COLS, R).T[::-1, :],
                                np.zeros((1, COLS), dtype=np.uint8)])

        xe_pad = np.zeros(128 * F_XE, dtype=ml_dtypes.bfloat16)
        sl = xe_all[c * per:(c + 1) * per]
        xe_pad[:len(sl)] = sl

        pm = np.zeros((1, NC), dtype=np.float32)
        pm[0, :c] = 1.0

        per_core.append({
            "xs_main": np.ascontiguousarray(
                xcols[:, :T_FULL * FT].reshape(128, T_FULL, FT).transpose(1, 0, 2)),
            "xs_tail": np.ascontiguousarray(xcols[:, T_FULL * FT:]),
            "cm_main": np.ascontiguousarray(
                ccols[:, :T_FULL * FT].reshape(128, T_FULL, FT).transpose(1, 0, 2)),
            "cm_tail": np.ascontiguousarray(ccols[:, T_FULL * FT:]),
            "xe": xe_pad.reshape(128, F_XE),
            "prefmask": pm,
        })
    return per_core


LAST_EXEC_NS = {}


def kernel(x, times, events):
    per_core = _prepare(x, times, events)
    trace = bool(int(os.environ.get("BASS_COX_TRACE", "0")))
    nc = _get("v3", _build)
    res = run_bass_kernel_spmd(nc, per_core, core_ids=list(range(NC)),
                               trace=trace)
    LAST_EXEC_NS.clear()
    LAST_EXEC_NS["b"] = res.exec_time_ns

    a_tot = 0.0
    b_tot = 0.0
    for c in range(NC):
        ab = res.results[c]["ab"]
        a_tot += float(ab[0, 0])
        b_tot += float(ab[1, 0])
    loss = np.sqrt((b_tot - a_tot) / N)
    return np.float32(loss)


# revision 23
# speedup vs baseline: 1.1834x; 1.1117x over previous
"""Cox proportional-hazards loss (Breslow ties, sqrt of mean) on 8 trn2 cores.

Math: sort by descending time; with e = exp(x), Q_j = global inclusive prefix
sum of e and, for each tied-time segment end j, C_j = event count of that
segment:
    loss = sqrt(( sum_{ends j} C_j * ln(Q_j)  -  sum_i ev_i * x_i ) / N)

Device algorithm (v3, PE-centric). The host does layout/ordering and integer
mask/count bookkeeping only (argsort, gather, tie detection, integer event
counts per segment); every floating-point reduction over the data runs on
device:
  - Records are laid out column-major: each SBUF column holds 127 consecutive
    records (rows 126..0, reversed) plus a per-column carry injected into row
    127. One upper-triangular [128,128] matmul per 512-column chunk then
    yields the inclusive prefix Q for all records of the chunk at once, with
    the cross-column carry folded in by the always-included row 127.
  - Cross-column carries: per-column sums (ones-vector matmuls) are staged to
    DRAM, reloaded as [128,130] (130 columns per partition), prefix-scanned
    along the free dim (one cheap DVE scan), closed over partitions with a
    strict-lower-triangular matmul, and written back as the row-127 carries.
  - The cross-core carry (sum of exp over all previous cores) is obtained
    with an in-kernel AllReduce of the 8 per-core totals and enters as the
    per-partition bias of the Ln activation: lnQ = Ln(Q_psum + core_offset).
  - b-term: one scalar_tensor_tensor per half-tile accumulates
    sum(cm * lnQ), where cm is the host-provided integer count stream
    (cm[j] = segment event count if j is a segment end else 0).
  - a-term: sum(ev * x) = sum of the host-gathered event-score stream xe
    (gather is layout work, like argsort), chunk-reduced on DVE.
  - x ships as fp8e4m3 (loss sits in ln of ~16M-term sums; quantization error
    on the final loss is ~1e-5, tolerance 2e-2); flags/counts as uint8.
"""

import os
import sys

for _p in ("/opt/trn_rl_repo", "/root/.axon_site/_ro/trn_rl_repo"):
    if os.path.isdir(_p) and _p not in sys.path:
        sys.path.insert(0, _p)

import numpy as np
import ml_dtypes

import concourse.bass as bass
import concourse.tile as tile
from concourse import bacc, mybir
from concourse.bass_utils import run_bass_kernel_spmd

# Pin the activation table that contains both Exp and Ln so the compiler's
# table-selection pass never thrashes ACT_TABLE_LOADs between them.
import concourse.hw_specs as _hw_specs

_orig_get_tables = _hw_specs.get_activation_tables


def _single_table(arch):
    tabs = dict(_orig_get_tables(arch))
    keep = "natural_log_exp_and_others"
    return {name: (s if name == keep else set()) for name, s in tabs.items()}


bacc.get_activation_tables = _single_table

N = 16777216
NC = 8
NLOC = N // NC            # 2097152 records per core
R = 127                   # records per column (row 127 holds the carry)
COLS = -(-NLOC // R)      # 16514 columns per core
FT = 2048                 # columns per full tile
T_FULL = COLS // FT       # 8 full tiles
F_TAIL = COLS - T_FULL * FT   # 130 columns in the tail tile
CCAP = 130                # per-partition columns in the carry chain (128*130 >= COLS)
F_XE = 8448               # event-score stream: [128, F_XE] per core

_DT = mybir.dt
_ACT = mybir.ActivationFunctionType
_ALU = mybir.AluOpType


def _build(debug=False):
    nc = bacc.Bacc("TRN2", target_bir_lowering=False, debug=False, num_devices=NC)
    xs_main = nc.dram_tensor("xs_main", [T_FULL, 128, FT], _DT.float8e4,
                             kind="ExternalInput")
    xs_tail = nc.dram_tensor("xs_tail", [128, F_TAIL], _DT.float8e4,
                             kind="ExternalInput")
    cm_main = nc.dram_tensor("cm_main", [T_FULL, 128, FT], _DT.uint8,
                             kind="ExternalInput")
    cm_tail = nc.dram_tensor("cm_tail", [128, F_TAIL], _DT.uint8,
                             kind="ExternalInput")
    xe_in = nc.dram_tensor("xe", [128, F_XE], _DT.bfloat16, kind="ExternalInput")
    prefmask = nc.dram_tensor("prefmask", [1, NC], _DT.float32,
                              kind="ExternalInput")
    ab_out = nc.dram_tensor("ab", [2, 1], _DT.float32, kind="ExternalOutput")
    if debug:
        cs_dbg = nc.dram_tensor("cs_dbg", [128, CCAP], _DT.float32,
                                kind="ExternalOutput")
        carr_dbg = nc.dram_tensor("carr_dbg", [128, CCAP], _DT.float32,
                                  kind="ExternalOutput")
        lnq_dbg = nc.dram_tensor("lnq_dbg", [R, FT], _DT.float32,
                                 kind="ExternalOutput")

    FS = [FT] * T_FULL + [F_TAIL]
    NT = len(FS)

    with tile.TileContext(nc) as tc:
        with (
            tc.tile_pool(name="io", bufs=3) as io,
            tc.tile_pool(name="wk", bufs=3) as wk,
            tc.tile_pool(name="keep", bufs=1) as keep,
            tc.tile_pool(name="sm", bufs=1) as sm,
            tc.tile_pool(name="psq", bufs=2, space="PSUM") as psq,
            tc.tile_pool(name="psc", bufs=2, space="PSUM") as psc,
            tc.tile_pool(name="pss", bufs=1, space="PSUM") as pss,
            tc.tile_pool(name="dram", bufs=1, space="DRAM") as dram,
        ):
            # ---- constants -------------------------------------------------
            # ut[p, po] = 1 iff po <= p  (inclusive suffix over partitions:
            # out[po] = sum_{p >= po} in[p]); row 127 (carry) is in every sum.
            ut = sm.tile([128, 128], _DT.bfloat16)
            nc.gpsimd.memset(ut[:], 1.0)
            # keep iff 1 + p - po > 0  <=>  po <= p
            nc.gpsimd.affine_select(
                ut[:], ut[:], pattern=[[-1, 128]], compare_op=_ALU.is_gt,
                fill=0.0, base=1, channel_multiplier=1)
            # strict lower triangular (as lhsT): out[po] = sum_{p < po} in[p]
            ltri = sm.tile([128, 128], _DT.float32)
            nc.gpsimd.memset(ltri[:], 1.0)
            nc.gpsimd.affine_select(
                ltri[:], ltri[:], pattern=[[1, 128]], compare_op=_ALU.is_gt,
                fill=0.0, base=0, channel_multiplier=-1)
            ones127 = sm.tile([R, 1], _DT.bfloat16)
            nc.gpsimd.memset(ones127[:], 1.0)
            ones127f = sm.tile([R, 1], _DT.float32)
            nc.gpsimd.memset(ones127f[:], 1.0)
            ones_col = sm.tile([128, 1], _DT.float32)
            nc.gpsimd.memset(ones_col[:], 1.0)
            ones_row = sm.tile([1, 128], _DT.float32)
            nc.gpsimd.memset(ones_row[:], 1.0)
            zeros_sm = sm.tile([1, 128], _DT.float32)
            nc.gpsimd.memset(zeros_sm[:], 0.0)

            pref_sb = sm.tile([1, NC], _DT.float32)
            nc.sync.dma_start(pref_sb[:], prefmask.ap())

            colsum_dram = dram.tile([1, 128 * CCAP], _DT.float32)
            carr_dram = dram.tile([1, 128 * CCAP], _DT.bfloat16)
            cc_in = dram.tile([1, NC], _DT.float32)
            cc_out = dram.tile([1, NC], _DT.float32)

            acc_b = sm.tile([R, 2 * T_FULL + 1], _DT.float32)
            s_all = sm.tile([R, NT], _DT.float32)
            a_p = sm.tile([128, 1], _DT.float32)
            pssT = pss.tile([128, 4], _DT.float32)

            # zero the unused tail of the colsum staging area
            nc.sync.dma_start(colsum_dram[:, COLS:128 * CCAP],
                              zeros_sm[:, 0:128 * CCAP - COLS])

            # ---- phase A: exp + per-column sums ---------------------------
            # Column-sum chunks (512 cols, rows 0..126 via ones matmul) pack
            # three at a time onto partitions {0,32,64} of a 1-bank psum
            # tile; each full generation is copied to SBUF once (gpsimd) and
            # row-DMA'd to the DRAM staging vector.
            e_tiles = []
            pend = []        # (psum_row, global_col_start, width)
            psC = [None]

            def _flush():
                if not pend:
                    return
                cs = wk.tile([128, 512], _DT.float32)
                nc.vector.tensor_scalar_add(cs[:], psC[0][:], 0.0)
                for row, g0, w in pend:
                    nc.sync.dma_start(colsum_dram[:, g0:g0 + w],
                                      cs[row:row + 1, 0:w])
                pend.clear()
                psC[0] = None

            for t in range(NT):
                F = FS[t]
                xs = io.tile([128, F], _DT.float8e4)
                if t < T_FULL:
                    nc.sync.dma_start(xs[:], xs_main.ap()[t])
                else:
                    nc.sync.dma_start(xs[:], xs_tail.ap())
                e = keep.tile([128, F], _DT.bfloat16, name=f"e_{t}")
                nc.scalar.activation(e[0:R, :], xs[0:R, :], _ACT.Exp,
                                     accum_out=s_all[:, t:t + 1])
                e_tiles.append(e)

                for j in range(-(-F // 512)):
                    c0, c1 = 512 * j, min(512 * (j + 1), F)
                    if psC[0] is None:
                        psC[0] = psc.tile([128, 512], _DT.float32,
                                          name="psC_gen")
                    row = 32 * len(pend)
                    nc.tensor.matmul(psC[0][row:row + 1, 0:c1 - c0],
                                     ones127[:], e[0:R, c0:c1],
                                     start=True, stop=True)
                    pend.append((row, FT * t + c0, c1 - c0))
                    if len(pend) == 3:
                        _flush()
            _flush()

            # ---- per-core total (from exp accumulators) -> early AllGather
            s_p = sm.tile([R, 1], _DT.float32)
            nc.vector.tensor_reduce(s_p[:], s_all[:], mybir.AxisListType.X,
                                    _ALU.add)
            tot_ps = pssT[0:1, 1:2]
            nc.tensor.matmul(tot_ps, s_p[:], ones127f[:],
                             start=True, stop=True)
            tot_sb = sm.tile([1, 1], _DT.float32)
            nc.scalar.copy(tot_sb[:], tot_ps)
            nc.gpsimd.dma_start(cc_in[:, 0:1], tot_sb[:])
            nc.gpsimd.collective_compute(
                "AllGather", _ALU.bypass, replica_groups=[list(range(NC))],
                ins=[cc_in[:, 0:1].opt()], outs=[cc_out[:].opt()])

            # ---- carry chain (overlaps the collective) --------------------
            cs128 = keep.tile([128, CCAP], _DT.float32)
            nc.sync.dma_start(
                cs128[:], colsum_dram[:].rearrange("a (p j) -> (a p) j", p=128))
            incl = keep.tile([128, CCAP], _DT.float32)
            nc.vector.tensor_tensor_scan(incl[:], cs128[:], cs128[:], 0.0,
                                         _ALU.add, _ALU.bypass)
            # rest of the carry chain (overlaps the collective)
            rowc_ps = pssT[:, 0:1]
            nc.tensor.matmul(rowc_ps, ltri[:], incl[:, CCAP - 1:CCAP],
                             start=True, stop=True)
            carr = keep.tile([128, CCAP], _DT.bfloat16)
            nc.scalar.copy(carr[:, 0:1], rowc_ps)
            nc.vector.scalar_tensor_tensor(carr[:, 1:CCAP], incl[:, 0:CCAP - 1],
                                           rowc_ps, incl[:, 0:CCAP - 1],
                                           _ALU.add, _ALU.bypass)
            nc.sync.dma_start(
                carr_dram[:].rearrange("a (p j) -> (a p) j", p=128), carr[:])
            if debug:
                nc.sync.dma_start(cs_dbg.ap(), cs128[:])
                carr32 = keep.tile([128, CCAP], _DT.float32)
                nc.vector.tensor_scalar_add(carr32[:], carr[:], 0.0)
                nc.sync.dma_start(carr_dbg.ap(), carr32[:])
            # inject per-tile carry rows
            for t in range(NT):
                F = FS[t]
                nc.scalar.dma_start(e_tiles[t][127:128, :],
                                    carr_dram[:, FT * t:FT * t + F])

            # ---- a-term: sum of host-gathered event scores -----------------
            # (issued on the scalar queue; reduced on DVE while it waits for
            # the collective)
            xe = keep.tile([128, F_XE], _DT.bfloat16)
            nc.scalar.dma_start(xe[:], xe_in.ap())
            a_acc = sm.tile([128, 8], _DT.float32)
            xch = F_XE // 8
            for k in range(8):
                nc.vector.tensor_reduce(a_acc[:, k:k + 1],
                                        xe[:, xch * k:xch * (k + 1)],
                                        mybir.AxisListType.X, _ALU.add)
            nc.vector.tensor_reduce(a_p[:], a_acc[:], mybir.AxisListType.X,
                                    _ALU.add)

            # ---- cross-core bias (emitted late so the waiting copy does
            # not stall the scalar queue before injections/cm are out) ------
            allt = sm.tile([1, NC], _DT.float32)
            nc.gpsimd.dma_start(allt[:], cc_out[:])
            off = sm.tile([1, 1], _DT.float32)
            junk_o = sm.tile([1, NC], _DT.float32)
            nc.vector.scalar_tensor_tensor(junk_o[:], allt[:], 0.0, pref_sb[:],
                                           _ALU.bypass, _ALU.mult,
                                           accum_out=off[:])
            bias_ps = pssT[:, 2:3]
            nc.tensor.matmul(bias_ps, ones_row[:], off[:],
                             start=True, stop=True)
            bias = sm.tile([128, 1], _DT.float32)
            nc.scalar.copy(bias[:], bias_ps)

            # ---- phase B: Q prefix, lnQ, b-term ---------------------------
            for t in range(NT):
                F = FS[t]
                cm = io.tile([128, F], _DT.uint8)
                if t < T_FULL:
                    nc.sync.dma_start(cm[:], cm_main.ap()[t])
                else:
                    nc.sync.dma_start(cm[:], cm_tail.ap())
                e = e_tiles[t]
                nhalf = -(-F // 1024)
                for h in range(nhalf):
                    h0, h1 = 1024 * h, min(1024 * (h + 1), F)
                    psQ = psq.tile([128, 1024], _DT.float32)
                    for c0 in range(h0, h1, 512):
                        c1 = min(c0 + 512, h1)
                        nc.tensor.matmul(psQ[:, c0 - h0:c1 - h0], ut[:],
                                         e[:, c0:c1], start=True, stop=True)
                    lnq = wk.tile([R, 1024], _DT.bfloat16)
                    nc.scalar.activation(lnq[:, 0:h1 - h0], psQ[0:R, 0:h1 - h0],
                                         _ACT.Ln, bias=bias[0:R, 0:1])
                    junk = wk.tile([R, 1024], _DT.bfloat16)
                    nc.vector.scalar_tensor_tensor(
                        junk[:, 0:h1 - h0], cm[0:R, h0:h1], 0.0,
                        lnq[:, 0:h1 - h0], _ALU.bypass, _ALU.mult,
                        accum_out=acc_b[:, 2 * t + h:2 * t + h + 1])
                    if debug and t == 0:
                        lnq32 = wk.tile([R, 1024], _DT.float32)
                        nc.vector.tensor_scalar_add(lnq32[:, 0:h1 - h0],
                                                    lnq[:, 0:h1 - h0], 0.0)
                        nc.sync.dma_start(lnq_dbg.ap()[:, h0:h1],
                                          lnq32[:, 0:h1 - h0])

            # ---- combine --------------------------------------------------
            ab = sm.tile([128, 2], _DT.float32)
            nc.gpsimd.memset(ab[:], 0.0)
            nc.vector.tensor_reduce(ab[0:R, 1:2], acc_b[:],
                                    mybir.AxisListType.X, _ALU.add)
            nc.scalar.copy(ab[:, 0:1], a_p[:])
            ab_ps = pssT[0:2, 3:4]
            nc.tensor.matmul(ab_ps, ab[:], ones_col[:], start=True, stop=True)
            ab_sb = sm.tile([2, 1], _DT.float32)
            nc.scalar.copy(ab_sb[:], ab_ps)
            nc.sync.dma_start(ab_out.ap(), ab_sb[:])
    nc.compile()
    return nc


_CACHE = {}


def _get(name, builder):
    if name not in _CACHE:
        _CACHE[name] = builder()
    return _CACHE[name]


def _prepare(x, times, events):
    x = np.asarray(x, dtype=np.float32)
    times = np.asarray(times, dtype=np.int32)
    events = np.asarray(events, dtype=np.int32)
    assert x.shape == (N,)

    order = np.argsort(-times)           # descending time; tie order irrelevant
    xs = x[order]
    ts = times[order]
    ev = events[order].astype(np.int64)

    # integer bookkeeping: segment ends and per-segment event counts
    is_end = np.empty(N, dtype=bool)
    np.not_equal(ts[:-1], ts[1:], out=is_end[:-1])
    is_end[N - 1] = True
    endidx = np.flatnonzero(is_end)
    E = np.cumsum(ev)
    Eend = E[endidx]
    C = np.diff(np.concatenate([[0], Eend]))
    assert C.max() < 256
    cm = np.zeros(N, dtype=np.uint8)
    cm[endidx] = C

    # host-gathered event scores (a-term), split evenly across cores
    xe_all = x[events == 1].astype(ml_dtypes.bfloat16)
    EV = len(xe_all)
    per = -(-EV // NC)
    assert per <= 128 * F_XE

    xs8 = xs.astype(ml_dtypes.float8_e4m3fn)
    per_core = []
    for c in range(NC):
        cs = c * NLOC
        # column-major layout: column k holds records [cs+127k, cs+127k+127)
        # at partitions 126..0 (reversed); row 127 is the carry slot.
        xa = np.full(COLS * R, -100.0, dtype=ml_dtypes.float8_e4m3fn)
        xa[:NLOC] = xs8[cs:cs + NLOC]
        xcols = np.concatenate(
            [xa.reshape(COLS, R).T[::-1, :],
             np.full((1, COLS), -100.0,
                     dtype=ml_dtypes.float8_e4m3fn)])  # [128, COLS]
        ca = np.zeros(COLS * R, dtype=np.uint8)
        ca[:NLOC] = cm[cs:cs + NLOC]
        ccols = np.concatenate([ca.reshape(COLS, R).T[::-1, :],
                                np.zeros((1, COLS), dtype=np.uint8)])

        xe_pad = np.zeros(128 * F_XE, dtype=ml_dtypes.bfloat16)
        sl = xe_all[c * per:(c + 1) * per]
        xe_pad[:len(sl)] = sl

        pm = np.zeros((1, NC), dtype=np.float32)
        pm[0, :c] = 1.0

        per_core.append({
            "xs_main": np.ascontiguousarray(
                xcols[:, :T_FULL * FT].reshape(128, T_FULL, FT).transpose(1, 0, 2)),
            "xs_tail": np.ascontiguousarray(xcols[:, T_FULL * FT:]),
            "cm_main": np.ascontiguousarray(
                ccols[:, :T_FULL * FT].reshape(128, T_FULL, FT).transpose(1, 0, 2)),
            "cm_tail": np.ascontiguousarray(ccols[:, T_FULL * FT:]),
            "xe": xe_pad.reshape(128, F_XE),
            "prefmask": pm,
        })
    return per_core


LAST_EXEC_NS = {}


def kernel(x, times, events):
    per_core = _prepare(x, times, events)
    trace = bool(int(os.environ.get("BASS_COX_TRACE", "0")))
    nc = _get("v3", _build)
    res = run_bass_kernel_spmd(nc, per_core, core_ids=list(range(NC)),
                               trace=trace)
    LAST_EXEC_NS.clear()
    LAST_EXEC_NS["b"] = res.exec_time_ns

    a_tot = 0.0
    b_tot = 0.0
    for c in range(NC):
        ab = res.results[c]["ab"]
        a_tot += float(ab[0, 0])
        b_tot += float(ab[1, 0])
    loss = np.sqrt((b_tot - a_tot) / N)
    return np.float32(loss)
